# revision 6
# baseline (speedup 1.0000x reference)
"""MoE layer (8 experts, top-2) on 8 Trainium2 NeuronCores.

Strategy (expert-parallel with host-side routing):
  - Router (0.01% of FLOPs) runs on host in float64: logits = x @ Wr.T,
    softmax, top-2, normalized gate weights.
  - Tokens are gathered per expert on host; core e runs expert e's FFN
    (gelu MLP 1024 -> 4096 -> 1024, bf16 in / fp32 accum / bf16 out) over
    the tokens routed to it, padded to a common capacity.
  - Host scatter-adds the gate-weighted expert outputs back into the full
    [B, T, D] fp32 output.

Device kernel per core:
  H^T tiles = gelu(W1T.T @ XT)   (matmul1: K=D, M=F-tile, N=token-chunk)
  Y tiles   = H^T.T @ W2T        (matmul2: K=F, M=token-tile, N=D-half)
  Y is rounded to bf16 (matches the reference's bf16 expert output).
"""

import numpy as np
import ml_dtypes

import concourse.mybir as mybir
import concourse.tile as tile
from concourse import bacc
from concourse.bass_utils import run_bass_kernel_spmd

BF16 = ml_dtypes.bfloat16

B, T, D, F, E = 2, 2048, 1024, 4096, 8
N = B * T
TOP_K = 2
P = 128
KD = D // P   # k-tiles for matmul1
KF = F // P   # k-tiles for matmul2
NCHUNK = 512  # token-chunk (moving free dim of matmul1)


def _route(xf, Wr):
    """Top-2 routing in float64. Returns (idx [N,2], gates [N,2] fp32)."""
    logits = xf.astype(np.float64) @ Wr.T.astype(np.float64)
    logits -= logits.max(axis=-1, keepdims=True)
    p = np.exp(logits)
    p /= p.sum(axis=-1, keepdims=True)
    # top-2 (ties broken by lower index, same as jax.lax.top_k)
    order = np.argsort(-p, axis=-1, kind="stable")
    idx = order[:, :TOP_K]
    pw = np.take_along_axis(p, idx, axis=-1).astype(np.float32)
    gates = pw / pw.sum(axis=-1, keepdims=True)
    return idx, gates


def _chunks_of(c):
    """Split capacity c into chunks of at most NCHUNK, multiples of P."""
    out = []
    while c > 0:
        s = min(NCHUNK, c)
        out.append(s)
        c -= s
    return out


_PROGRAM_CACHE = {}


def _build_program(cap):
    """Build the SPMD Bass program for per-core capacity `cap` tokens."""
    if cap in _PROGRAM_CACHE:
        return _PROGRAM_CACHE[cap]

    bf = mybir.dt.bfloat16
    f32 = mybir.dt.float32

    nc = bacc.Bacc()
    xt_d = nc.declare_dram_parameter("xt", [D, cap], bf, isOutput=False)
    w1t_d = nc.declare_dram_parameter("w1t", [D, F], bf, isOutput=False)
    w2t_d = nc.declare_dram_parameter("w2t", [F, D], bf, isOutput=False)
    y_d = nc.declare_dram_parameter("y", [cap, D], bf, isOutput=True)

    chunks = _chunks_of(cap)

    with tile.TileContext(nc) as tc:
        with (
            tc.tile_pool(name="wpool", bufs=1) as wpool,
            tc.tile_pool(name="xpool", bufs=2) as xpool,
            tc.tile_pool(name="hpool", bufs=1) as hpool,
            tc.tile_pool(name="ypool", bufs=4) as ypool,
            tc.tile_pool(name="hpsum", bufs=2, space="PSUM") as hpsum,
            tc.tile_pool(name="ypsum", bufs=2, space="PSUM") as ypsum,
        ):
            # Resident weights. w1 is needed first; emit its DMAs first.
            w1_tiles = []
            for k in range(KD):
                w = wpool.tile([P, F], bf, tag=f"w1_{k}")
                nc.sync.dma_start(w[:], w1t_d[k * P:(k + 1) * P, :])
                w1_tiles.append(w)
            w2_tiles = []
            for k in range(KF):
                w = wpool.tile([P, D], bf, tag=f"w2_{k}")
                nc.sync.dma_start(w[:], w2t_d[k * P:(k + 1) * P, :])
                w2_tiles.append(w)

            off = 0
            for csz in chunks:
                xt_tiles = []
                for k in range(KD):
                    xt = xpool.tile([P, csz], bf, tag=f"xt_{k}")
                    nc.sync.dma_start(
                        xt[:], xt_d[k * P:(k + 1) * P, off:off + csz]
                    )
                    xt_tiles.append(xt)

                # matmul1 + gelu -> H^T chunk [F, csz] bf16
                ht_tiles = []
                for f in range(KF):
                    ps = hpsum.tile([P, csz], f32, tag="hps")
                    for k in range(KD):
                        nc.tensor.matmul(
                            ps[:],
                            w1_tiles[k][:, f * P:(f + 1) * P],
                            xt_tiles[k][:],
                            start=(k == 0),
                            stop=(k == KD - 1),
                        )
                    ht = hpool.tile([P, csz], bf, tag=f"ht_{f}")
                    nc.scalar.activation(
                        ht[:], ps[:], mybir.ActivationFunctionType.Gelu
                    )
                    ht_tiles.append(ht)

                # matmul2 -> Y chunk [csz, D] bf16
                for t_ in range(csz // P):
                    for dh in range(D // 512):
                        ps = ypsum.tile([P, 512], f32, tag="yps")
                        for kf in range(KF):
                            nc.tensor.matmul(
                                ps[:],
                                ht_tiles[kf][:, t_ * P:(t_ + 1) * P],
                                w2_tiles[kf][:, dh * 512:(dh + 1) * 512],
                                start=(kf == 0),
                                stop=(kf == KF - 1),
                            )
                        yt = ypool.tile([P, 512], bf, tag="y")
                        nc.vector.tensor_copy(yt[:], ps[:])
                        nc.sync.dma_start(
                            y_d[off + t_ * P:off + (t_ + 1) * P,
                                dh * 512:(dh + 1) * 512],
                            yt[:],
                        )
                off += csz

    nc.finalize()
    _PROGRAM_CACHE[cap] = nc
    return nc


def _prepare(x, Wr, W1, W2):
    """Host routing + per-core input construction. Returns
    (in_maps, expert_token_lists, gate_per_token_list, cap)."""
    xf = np.ascontiguousarray(x.reshape(N, D), dtype=np.float32)
    idx, gates = _route(xf, Wr)

    tok_lists, gate_lists = [], []
    for e in range(E):
        hits = idx == e                        # [N, 2], at most one True/row
        toks = np.nonzero(hits.any(axis=1))[0]
        g = gates[hits]                        # row-major -> aligned with toks
        tok_lists.append(toks)
        gate_lists.append(g.astype(np.float32))

    counts = [len(t) for t in tok_lists]
    cap = ((max(counts) + P - 1) // P) * P

    xb = xf.astype(BF16)
    W1b = W1.astype(BF16)
    W2b = W2.astype(BF16)

    in_maps = []
    for e in range(E):
        xe = np.zeros((cap, D), dtype=BF16)
        xe[:counts[e]] = xb[tok_lists[e]]
        in_maps.append({
            "xt": np.ascontiguousarray(xe.T),
            "w1t": np.ascontiguousarray(W1b[e].T),
            "w2t": np.ascontiguousarray(W2b[e].T),
        })
    return in_maps, tok_lists, gate_lists, cap


def kernel(x, Wr, W1, W2, _trace=False):
    x = np.asarray(x, dtype=np.float32)
    Wr = np.asarray(Wr, dtype=np.float32)
    W1 = np.asarray(W1, dtype=np.float32)
    W2 = np.asarray(W2, dtype=np.float32)

    in_maps, tok_lists, gate_lists, cap = _prepare(x, Wr, W1, W2)
    nc = _build_program(cap)

    res = run_bass_kernel_spmd(nc, in_maps, list(range(E)), trace=_trace)

    out = np.zeros((N, D), dtype=np.float32)
    for e in range(E):
        c = len(tok_lists[e])
        y = np.asarray(res.results[e]["y"][:c], dtype=np.float32)
        out[tok_lists[e]] += y * gate_lists[e][:, None]

    out = out.reshape(B, T, D)
    if _trace:
        return out, res
    return out


# revision 7
# speedup vs baseline: 1.1695x; 1.1695x over previous
"""MoE layer (8 experts, top-2) on 8 Trainium2 NeuronCores.

Strategy (expert-parallel with host-side routing):
  - Router (0.01% of FLOPs) runs on host in float64: logits = x @ Wr.T,
    softmax, top-2, normalized gate weights.
  - Tokens are gathered per expert on host; core e runs expert e's FFN
    (gelu MLP 1024 -> 4096 -> 1024, bf16 in / fp32 accum / bf16 out) over
    the tokens routed to it, padded to a common capacity.
  - Host scatter-adds the gate-weighted expert outputs back into the full
    [B, T, D] fp32 output.

Device kernel per core:
  H^T tiles = gelu(W1T.T @ XT)   (matmul1: K=D, M=F-tile, N=token-chunk)
  Y tiles   = H^T.T @ W2T        (matmul2: K=F, M=token-tile, N=D-half)
  Y is rounded to bf16 (matches the reference's bf16 expert output).

DMA order is tuned so the PE can start ~3us in: token chunk 0 first, then
W1T in f-major [128, 512] column blocks (one f-block unlocks 4 matmul1
output tiles), then W2T k-tiles (not needed until matmul2, ~55us in).
"""

import numpy as np
import ml_dtypes

import concourse.mybir as mybir
import concourse.tile as tile
from concourse import bacc
from concourse.bass_utils import run_bass_kernel_spmd

BF16 = ml_dtypes.bfloat16

B, T, D, F, E = 2, 2048, 1024, 4096, 8
N = B * T
TOP_K = 2
P = 128
KD = D // P    # 8  k-tiles for matmul1
KF = F // P    # 32 k-tiles for matmul2
FB = 512       # w1 column-block width (4 f-tiles)
NCHUNK = 512   # token-chunk (moving free dim of matmul1)


def _route(xf, Wr):
    """Top-2 routing in float64. Returns (idx [N,2], gates [N,2] fp32)."""
    logits = xf.astype(np.float64) @ Wr.T.astype(np.float64)
    logits -= logits.max(axis=-1, keepdims=True)
    p = np.exp(logits)
    p /= p.sum(axis=-1, keepdims=True)
    # top-2 (ties broken by lower index, same as jax.lax.top_k)
    order = np.argsort(-p, axis=-1, kind="stable")
    idx = order[:, :TOP_K]
    pw = np.take_along_axis(p, idx, axis=-1).astype(np.float32)
    gates = pw / pw.sum(axis=-1, keepdims=True)
    return idx, gates


def _chunks_of(c):
    """Split capacity c into chunks of at most NCHUNK."""
    out = []
    while c > 0:
        s = min(NCHUNK, c)
        out.append(s)
        c -= s
    return out


def _token_tiles(csz):
    """Split a chunk into matmul2 output-partition tiles of at most P."""
    out = []
    while csz > 0:
        s = min(P, csz)
        out.append(s)
        csz -= s
    return out


_PROGRAM_CACHE = {}


def _build_program(cap, cap_pad):
    """SPMD Bass program: capacity `cap` tokens, DRAM padded to cap_pad."""
    key = (cap, cap_pad)
    if key in _PROGRAM_CACHE:
        return _PROGRAM_CACHE[key]

    bf = mybir.dt.bfloat16
    f32 = mybir.dt.float32

    nc = bacc.Bacc()
    xt_d = nc.declare_dram_parameter("xt", [D, cap_pad], bf, isOutput=False)
    w1t_d = nc.declare_dram_parameter("w1t", [D, F], bf, isOutput=False)
    w2t_d = nc.declare_dram_parameter("w2t", [F, D], bf, isOutput=False)
    y_d = nc.declare_dram_parameter("y", [cap_pad, D], bf, isOutput=True)

    chunks = _chunks_of(cap)

    with tile.TileContext(nc) as tc:
        with (
            tc.tile_pool(name="wpool", bufs=1) as wpool,
            tc.tile_pool(name="xpool", bufs=2) as xpool,
            tc.tile_pool(name="hpool", bufs=1) as hpool,
            tc.tile_pool(name="ypool", bufs=4) as ypool,
            tc.tile_pool(name="hpsum", bufs=3, space="PSUM") as hpsum,
            tc.tile_pool(name="ypsum", bufs=3, space="PSUM") as ypsum,
        ):
            # Token chunk 0 first: PE's first matmul needs it.
            xt_tiles0 = []
            for k in range(KD):
                xt = xpool.tile([P, chunks[0]], bf, tag=f"xt_{k}")
                nc.sync.dma_start(xt[:], xt_d[k * P:(k + 1) * P, :chunks[0]])
                xt_tiles0.append(xt)

            # W1T as f-major column blocks: w1_tiles[fb][k] is
            # w1t[k*128:(k+1)*128, fb*512:(fb+1)*512]. One block (8 DMAs,
            # 1MB) unlocks 4 matmul1 output tiles.
            w1_tiles = []
            for fb in range(F // FB):
                blk = []
                for k in range(KD):
                    w = wpool.tile([P, FB], bf, tag=f"w1_{fb}_{k}")
                    nc.sync.dma_start(
                        w[:], w1t_d[k * P:(k + 1) * P, fb * FB:(fb + 1) * FB]
                    )
                    blk.append(w)
                w1_tiles.append(blk)

            # W2T k-tiles in matmul2 accumulation order.
            w2_tiles = []
            for k in range(KF):
                w = wpool.tile([P, D], bf, tag=f"w2_{k}")
                nc.sync.dma_start(w[:], w2t_d[k * P:(k + 1) * P, :])
                w2_tiles.append(w)

            off = 0
            for ci, csz in enumerate(chunks):
                if ci == 0:
                    xt_tiles = xt_tiles0
                else:
                    xt_tiles = []
                    for k in range(KD):
                        xt = xpool.tile([P, csz], bf, tag=f"xt_{k}")
                        nc.sync.dma_start(
                            xt[:], xt_d[k * P:(k + 1) * P, off:off + csz]
                        )
                        xt_tiles.append(xt)

                # matmul1 + gelu -> H^T chunk [F, csz] bf16
                ht_tiles = []
                for f in range(KF):
                    ps = hpsum.tile([P, csz], f32, tag="hps")
                    w1blk = w1_tiles[f // 4]
                    fo = (f % 4) * P
                    for k in range(KD):
                        nc.tensor.matmul(
                            ps[:],
                            w1blk[k][:, fo:fo + P],
                            xt_tiles[k][:],
                            start=(k == 0),
                            stop=(k == KD - 1),
                        )
                    ht = hpool.tile([P, csz], bf, tag=f"ht_{f}")
                    nc.scalar.activation(
                        ht[:], ps[:], mybir.ActivationFunctionType.Gelu
                    )
                    ht_tiles.append(ht)

                # matmul2 -> Y chunk [csz, D] bf16
                to = 0
                for tsz in _token_tiles(csz):
                    for dh in range(D // 512):
                        ps = ypsum.tile([P, 512], f32, tag="yps")
                        for kf in range(KF):
                            nc.tensor.matmul(
                                ps[:tsz, :],
                                ht_tiles[kf][:, to:to + tsz],
                                w2_tiles[kf][:, dh * 512:(dh + 1) * 512],
                                start=(kf == 0),
                                stop=(kf == KF - 1),
                            )
                        yt = ypool.tile([P, 512], bf, tag="y")
                        nc.vector.tensor_copy(yt[:tsz, :], ps[:tsz, :])
                        nc.sync.dma_start(
                            y_d[off + to:off + to + tsz,
                                dh * 512:(dh + 1) * 512],
                            yt[:tsz, :],
                        )
                    to += tsz
                off += csz

    nc.finalize()
    _PROGRAM_CACHE[key] = nc
    return nc


def _prepare(x, Wr, W1, W2):
    """Host routing + per-core input construction."""
    xf = np.ascontiguousarray(x.reshape(N, D), dtype=np.float32)
    idx, gates = _route(xf, Wr)

    tok_lists, gate_lists = [], []
    for e in range(E):
        hits = idx == e                        # [N, 2], at most one True/row
        toks = np.nonzero(hits.any(axis=1))[0]
        g = gates[hits]                        # row-major -> aligned with toks
        tok_lists.append(toks)
        gate_lists.append(g.astype(np.float32))

    counts = [len(t) for t in tok_lists]
    cap = max(counts)
    cap_pad = ((cap + P - 1) // P) * P

    xb = xf.astype(BF16)
    W1b = W1.astype(BF16)
    W2b = W2.astype(BF16)

    in_maps = []
    for e in range(E):
        xe = np.zeros((cap_pad, D), dtype=BF16)
        xe[:counts[e]] = xb[tok_lists[e]]
        in_maps.append({
            "xt": np.ascontiguousarray(xe.T),
            "w1t": np.ascontiguousarray(W1b[e].T),
            "w2t": np.ascontiguousarray(W2b[e].T),
        })
    return in_maps, tok_lists, gate_lists, cap, cap_pad


def kernel(x, Wr, W1, W2, _trace=False):
    x = np.asarray(x, dtype=np.float32)
    Wr = np.asarray(Wr, dtype=np.float32)
    W1 = np.asarray(W1, dtype=np.float32)
    W2 = np.asarray(W2, dtype=np.float32)

    in_maps, tok_lists, gate_lists, cap, cap_pad = _prepare(x, Wr, W1, W2)
    nc = _build_program(cap, cap_pad)

    res = run_bass_kernel_spmd(nc, in_maps, list(range(E)), trace=_trace)

    out = np.zeros((N, D), dtype=np.float32)
    for e in range(E):
        c = len(tok_lists[e])
        y = np.asarray(res.results[e]["y"][:c], dtype=np.float32)
        out[tok_lists[e]] += y * gate_lists[e][:, None]

    out = out.reshape(B, T, D)
    if _trace:
        return out, res
    return out


# revision 16
# speedup vs baseline: 1.2059x; 1.0311x over previous
"""MoE layer (8 experts, top-2) on 8 Trainium2 NeuronCores.

Strategy (expert-parallel with host-side routing):
  - Router (0.01% of FLOPs) runs on host in float64: logits = x @ Wr.T,
    softmax, top-2, normalized gate weights.
  - Tokens are gathered per expert on host; core e runs expert e's FFN
    (gelu MLP 1024 -> 4096 -> 1024, bf16 in / fp32 accum / bf16 out) over
    the tokens routed to it, padded to the max expert load (the SPMD
    program is shared, so every core runs the max-count shape).
  - Host scatter-adds the gate-weighted expert outputs back into the full
    [B, T, D] fp32 output.

Device kernel per core (tokens processed in chunks of <=512):
  H^T tiles = gelu(W1T.T @ XT)   (matmul1: K=D, M=F-tile, N=token-chunk)
  Y^T tiles = W2T.T @ H^T        (matmul2: K=F, M=D-tile, N=token-chunk)
  Y is rounded to bf16 (matches the reference's bf16 expert output) and
  returned transposed [D, cap]; the host transposes back.

Performance notes:
  - DMA throughput is governed by the contiguous bytes per SBUF partition
    row (packet size): [128, 4096] bf16 tiles give 8KB packets (~300GB/s
    aggregate), [128, 512] only 1KB (~130GB/s). Inputs are re-laid-out on
    host so every load has >=2KB rows.
  - W1 is loaded as f-quarter x k-half tiles (folded on host to 8KB rows),
    so matmul1's first 8 f-tiles only need ~2.1MB before the PE can start
    (~6us) instead of waiting for all 8.4MB of W1.
  - The 47-token remainder chunk's matmul1 is emitted before the last big
    chunk so its ACT-bound gelu tail hides under matmul work.
  - matmul2 produces Y^T (d on partitions, tokens moving): the final
    partial token tile costs 47 columns instead of a 37%-filled PE pass.
"""

import numpy as np
import ml_dtypes

import concourse.mybir as mybir
import concourse.tile as tile
from concourse import bacc
from concourse.bass_utils import run_bass_kernel_spmd

BF16 = ml_dtypes.bfloat16

B, T, D, F, E = 2, 2048, 1024, 4096, 8
N = B * T
TOP_K = 2
P = 128
KD = D // P    # 8  k-tiles for matmul1
KF = F // P    # 32 k-tiles for matmul2
ND = D // P    # 8  output d-tiles for matmul2
NCHUNK = 512   # max token-chunk (moving free dim)


def _route(xf, Wr):
    """Top-2 routing in float64. Returns (idx [N,2], gates [N,2] fp32)."""
    logits = xf.astype(np.float64) @ Wr.T.astype(np.float64)
    logits -= logits.max(axis=-1, keepdims=True)
    p = np.exp(logits)
    p /= p.sum(axis=-1, keepdims=True)
    # top-2 (ties broken by lower index, same as jax.lax.top_k)
    order = np.argsort(-p, axis=-1, kind="stable")
    idx = order[:, :TOP_K]
    pw = np.take_along_axis(p, idx, axis=-1).astype(np.float32)
    gates = pw / pw.sum(axis=-1, keepdims=True)
    return idx, gates


def _chunks_of(cap):
    """Full NCHUNK chunks plus a remainder, e.g. 1071 -> [512, 512, 47]."""
    out = []
    c = cap
    while c > 0:
        s = min(NCHUNK, c)
        out.append(s)
        c -= s
    return out


_PROGRAM_CACHE = {}


def _build_program(cap):
    if cap in _PROGRAM_CACHE:
        return _PROGRAM_CACHE[cap]

    bf = mybir.dt.bfloat16
    f32 = mybir.dt.float32

    chunks = _chunks_of(cap)
    # processing (and xt layout) order: big chunk 0, remainder, rest
    order = [0] + list(range(len(chunks) - 1, 0, -1)) + []
    order = [0, len(chunks) - 1] + list(range(1, len(chunks) - 1))
    csizes = [chunks[i] for i in order]
    # token offset of each processed chunk within the gathered sequence
    starts = np.concatenate([[0], np.cumsum(chunks)[:-1]])
    offs = [int(starts[i]) for i in order]

    nc = bacc.Bacc()
    # xt: folded [128, 8*cap]; processed chunk j occupies columns
    # [8*sum(csizes[:j]), ...), free index inside a chunk = k*csz + c.
    xt_d = nc.declare_dram_parameter("xt", [P, KD * cap], bf, isOutput=False)
    w1t_d = nc.declare_dram_parameter("w1t", [D, F], bf, isOutput=False)
    # w2 folded [1024, 4096]; row block m holds kf=4m..4m+3, free = kk*D + d.
    w2t_d = nc.declare_dram_parameter("w2t", [D, F], bf, isOutput=False)
    y_d = nc.declare_dram_parameter("y", [D, cap], bf, isOutput=True)

    with tile.TileContext(nc) as tc:
        with (
            tc.tile_pool(name="wpool", bufs=1) as wpool,
            tc.tile_pool(name="xpool", bufs=1) as xpool,
            tc.tile_pool(name="hpool", bufs=1) as hpool,
            tc.tile_pool(name="ypool", bufs=4) as ypool,
            tc.tile_pool(name="psum", bufs=8, space="PSUM") as psum,
        ):
            # ---- input DMAs, in the order the PE needs them ----
            xt_tiles = []
            xoff = 0
            for j, csz in enumerate(csizes):
                xt = xpool.tile([P, KD * csz], bf, tag=f"xt_{j}")
                # 4 parallel sub-DMAs (k-pairs) so the load spreads queues
                for s in range(4):
                    nc.sync.dma_start(
                        xt[:, s * 2 * csz:(s + 1) * 2 * csz],
                        xt_d[:, xoff + s * 2 * csz:xoff + (s + 1) * 2 * csz],
                    )
                xt_tiles.append(xt)
                xoff += KD * csz
                if j == 0:
                    # w1 folded tiles: block b = q*2 + h covers f-quarter q,
                    # k-half h; free index = (k%4)*1024 + (f%8)*128 + col.
                    w1_tiles = []
                    for b in range(8):
                        w = wpool.tile([P, F], bf, tag=f"w1_{b}")
                        nc.sync.dma_start(w[:], w1t_d[b * P:(b + 1) * P, :])
                        w1_tiles.append(w)
            w2_tiles = []
            for m in range(KF // 4):
                w = wpool.tile([P, 4 * D], bf, tag=f"w2_{m}")
                nc.sync.dma_start(w[:], w2t_d[m * P:(m + 1) * P, :])
                w2_tiles.append(w)

            def w1_slice(f, k):
                b = (f // 8) * 2 + k // 4
                o = (k % 4) * D + (f % 8) * P
                return w1_tiles[b][:, o:o + P]

            def w2_slice(kf, d):
                o = (kf % 4) * D + d * P
                return w2_tiles[kf // 4][:, o:o + P]

            def mm1_chunk(j, csz, tag):
                """matmul1 + gelu for one chunk -> 32 H^T tiles [128, csz]."""
                ht_tiles = [None] * KF
                for f in range(KF):
                    ps = psum.tile([P, NCHUNK], f32, tag="ps")
                    for k in range(KD):
                        nc.tensor.matmul(
                            ps[:, :csz],
                            w1_slice(f, k),
                            xt_tiles[j][:, k * csz:(k + 1) * csz],
                            start=(k == 0),
                            stop=(k == KD - 1),
                        )
                    ht = hpool.tile([P, csz], bf, tag=f"{tag}_{f}")
                    nc.scalar.activation(
                        ht[:], ps[:, :csz],
                        mybir.ActivationFunctionType.Gelu,
                    )
                    ht_tiles[f] = ht
                return ht_tiles

            def mm2_chunk(ht_tiles, csz, off):
                """matmul2 (Y^T) + copy + store for one chunk."""
                for d in range(ND):
                    ps = psum.tile([P, NCHUNK], f32, tag="ps")
                    for kf in range(KF):
                        nc.tensor.matmul(
                            ps[:, :csz],
                            w2_slice(kf, d),
                            ht_tiles[kf][:],
                            start=(kf == 0),
                            stop=(kf == KF - 1),
                        )
                    yt = ypool.tile([P, NCHUNK], bf, tag="y")
                    nc.vector.tensor_copy(yt[:, :csz], ps[:, :csz])
                    nc.sync.dma_start(
                        y_d[d * P:(d + 1) * P, off:off + csz], yt[:, :csz]
                    )

            ht0 = mm1_chunk(0, csizes[0], "ht")
            mm2_chunk(ht0, csizes[0], offs[0])
            if len(csizes) > 2:
                # remainder chunk's mm1 early: its gelu tail (ACT-bound)
                # hides under the following big chunk's matmul work
                hts = mm1_chunk(1, csizes[1], "hts")
                ht1 = mm1_chunk(2, csizes[2], "ht")
                mm2_chunk(hts, csizes[1], offs[1])
                mm2_chunk(ht1, csizes[2], offs[2])
                rest = range(3, len(csizes))
            else:
                rest = range(1, len(csizes))
            for j in rest:
                htj = mm1_chunk(j, csizes[j], "ht")
                mm2_chunk(htj, csizes[j], offs[j])

    nc.finalize()
    _PROGRAM_CACHE[cap] = (nc, order)
    return _PROGRAM_CACHE[cap]


def _fold_xt(xt_full, chunks, order):
    """[D, cap] -> [128, 8*cap] per-chunk (k, c) layout, processing order."""
    starts = np.concatenate([[0], np.cumsum(chunks)[:-1]])
    blocks = []
    for i in order:
        off, csz = int(starts[i]), chunks[i]
        blk = xt_full[:, off:off + csz]                   # [1024, csz]
        blk = blk.reshape(KD, P, csz).transpose(1, 0, 2).reshape(P, KD * csz)
        blocks.append(blk)
    return np.ascontiguousarray(np.concatenate(blocks, axis=1))


def _fold_w1(w1t):
    """[D, F] -> [1024, 4096]; row block b = q*2 + h holds f-quarter q,
    k-half h, free index (k%4)*1024 + (f%8)*128 + col."""
    # w1t[k*128+p, f] -> out[(q*2+h)*128+p, kk*1024 + j*128 + c]
    a = w1t.reshape(2, 4, P, 4, 8, P)       # [h, kk, p, q, j, c]
    a = a.transpose(3, 0, 2, 1, 4, 5)       # [q, h, p, kk, j, c]
    return np.ascontiguousarray(a.reshape(D, F))


def _fold_w2(w2t):
    """[F, D] -> [1024, 4096]; row block m holds kf=4m..4m+3."""
    return np.ascontiguousarray(
        w2t.reshape(KF // 4, 4, P, D).transpose(0, 2, 1, 3).reshape(D, F)
    )


def _prepare(x, Wr, W1, W2):
    xf = np.ascontiguousarray(x.reshape(N, D), dtype=np.float32)
    idx, gates = _route(xf, Wr)

    tok_lists, gate_lists = [], []
    for e in range(E):
        hits = idx == e                        # [N, 2], at most one True/row
        toks = np.nonzero(hits.any(axis=1))[0]
        g = gates[hits]                        # row-major -> aligned with toks
        tok_lists.append(toks)
        gate_lists.append(g.astype(np.float32))

    counts = [len(t) for t in tok_lists]
    cap = max(counts)
    return tok_lists, gate_lists, counts, cap, xf


def kernel(x, Wr, W1, W2, _trace=False):
    x = np.asarray(x, dtype=np.float32)
    Wr = np.asarray(Wr, dtype=np.float32)
    W1 = np.asarray(W1, dtype=np.float32)
    W2 = np.asarray(W2, dtype=np.float32)

    tok_lists, gate_lists, counts, cap, xf = _prepare(x, Wr, W1, W2)
    nc, order = _build_program(cap)
    chunks = _chunks_of(cap)

    xb = xf.astype(BF16)
    W1b = W1.astype(BF16)
    W2b = W2.astype(BF16)

    in_maps = []
    for e in range(E):
        xe = np.zeros((cap, D), dtype=BF16)
        xe[:counts[e]] = xb[tok_lists[e]]
        in_maps.append({
            "xt": _fold_xt(xe.T, chunks, order),
            "w1t": _fold_w1(np.ascontiguousarray(W1b[e].T)),
            "w2t": _fold_w2(np.ascontiguousarray(W2b[e].T)),
        })

    res = run_bass_kernel_spmd(nc, in_maps, list(range(E)), trace=_trace)

    out = np.zeros((N, D), dtype=np.float32)
    for e in range(E):
        c = counts[e]
        y = np.asarray(res.results[e]["y"][:, :c], dtype=np.float32)  # [D, c]
        out[tok_lists[e]] += y.T * gate_lists[e][:, None]

    out = out.reshape(B, T, D)
    if _trace:
        return out, res
    return out


# revision 21
# speedup vs baseline: 1.2080x; 1.0018x over previous
"""MoE layer (8 experts, top-2) on 8 Trainium2 NeuronCores.

Strategy (expert-parallel with host-side routing):
  - Router (0.01% of FLOPs) runs on host in float64: logits = x @ Wr.T,
    softmax, top-2, normalized gate weights.
  - Tokens are gathered per expert on host; core e runs expert e's FFN
    (gelu MLP 1024 -> 4096 -> 1024, bf16 in / fp32 accum / bf16 out) over
    the tokens routed to it, padded to the max expert load (the SPMD
    program is shared, so every core runs the max-count shape).
  - Host scatter-adds the gate-weighted expert outputs back into the full
    [B, T, D] fp32 output.

Device kernel per core (tokens processed in chunks of <=512):
  H^T tiles = gelu(W1T.T @ XT)   (matmul1: K=D, M=F-tile, N=token-chunk)
  Y^T tiles = W2T.T @ H^T        (matmul2: K=F, M=D-tile, N=token-chunk)
  Y is rounded to bf16 (matches the reference's bf16 expert output) and
  returned transposed [D, cap]; the host transposes back.

Performance notes:
  - DMA throughput is governed by the contiguous bytes per SBUF partition
    row (packet size): [128, 4096] bf16 tiles give 8KB packets (~300GB/s
    aggregate), [128, 512] only 1KB (~130GB/s). Inputs are re-laid-out on
    host so every load has >=2KB rows.
  - W1 is loaded as f-quarter x k-half tiles (folded on host to 8KB rows),
    so matmul1's first 8 f-tiles only need ~2.1MB before the PE can start
    (~6us) instead of waiting for all 8.4MB of W1.
  - The 47-token remainder chunk's matmul1 is emitted before the last big
    chunk so its ACT-bound gelu tail hides under matmul work.
  - matmul2 produces Y^T (d on partitions, tokens moving): the final
    partial token tile costs 47 columns instead of a 37%-filled PE pass.
"""

import numpy as np
import ml_dtypes

import concourse.mybir as mybir
import concourse.tile as tile
from concourse import bacc
from concourse.bass_utils import run_bass_kernel_spmd

BF16 = ml_dtypes.bfloat16

B, T, D, F, E = 2, 2048, 1024, 4096, 8
N = B * T
TOP_K = 2
P = 128
KD = D // P    # 8  k-tiles for matmul1
KF = F // P    # 32 k-tiles for matmul2
ND = D // P    # 8  output d-tiles for matmul2
NCHUNK = 512   # max token-chunk (moving free dim)


def _route(xf, Wr):
    """Top-2 routing in float64. Returns (idx [N,2], gates [N,2] fp32)."""
    logits = xf.astype(np.float64) @ Wr.T.astype(np.float64)
    logits -= logits.max(axis=-1, keepdims=True)
    p = np.exp(logits)
    p /= p.sum(axis=-1, keepdims=True)
    # top-2 (ties broken by lower index, same as jax.lax.top_k)
    order = np.argsort(-p, axis=-1, kind="stable")
    idx = order[:, :TOP_K]
    pw = np.take_along_axis(p, idx, axis=-1).astype(np.float32)
    gates = pw / pw.sum(axis=-1, keepdims=True)
    return idx, gates


def _chunks_of(cap):
    """Full NCHUNK chunks plus a remainder, e.g. 1071 -> [512, 512, 47]."""
    out = []
    c = cap
    while c > 0:
        s = min(NCHUNK, c)
        out.append(s)
        c -= s
    return out


_PROGRAM_CACHE = {}


def _build_program(cap):
    if cap in _PROGRAM_CACHE:
        return _PROGRAM_CACHE[cap]

    bf = mybir.dt.bfloat16
    f32 = mybir.dt.float32

    chunks = _chunks_of(cap)
    # processing (and xt layout) order: big chunk 0, remainder, rest
    order = [0] + list(range(len(chunks) - 1, 0, -1)) + []
    order = [0, len(chunks) - 1] + list(range(1, len(chunks) - 1))
    csizes = [chunks[i] for i in order]
    # token offset of each processed chunk within the gathered sequence
    starts = np.concatenate([[0], np.cumsum(chunks)[:-1]])
    offs = [int(starts[i]) for i in order]

    nc = bacc.Bacc()
    # xt: folded [128, 8*cap]; processed chunk j occupies columns
    # [8*sum(csizes[:j]), ...), free index inside a chunk = k*csz + c.
    xt_d = nc.declare_dram_parameter("xt", [P, KD * cap], bf, isOutput=False)
    w1t_d = nc.declare_dram_parameter("w1t", [D, F], bf, isOutput=False)
    # w2 folded [1024, 4096]; row block m holds kf=4m..4m+3, free = kk*D + d.
    w2t_d = nc.declare_dram_parameter("w2t", [D, F], bf, isOutput=False)
    # y folded like xt: chunk j at columns [8*xoff_j, ...), free index
    # inside a chunk = d_tile*csz + c (host unfolds).
    y_d = nc.declare_dram_parameter("y", [P, ND * cap], bf, isOutput=True)

    with tile.TileContext(nc) as tc:
        with (
            tc.tile_pool(name="wpool", bufs=1) as wpool,
            tc.tile_pool(name="xpool", bufs=1) as xpool,
            tc.tile_pool(name="hpool", bufs=1) as hpool,
            tc.tile_pool(name="ypool", bufs=4) as ypool,
            tc.tile_pool(name="psum", bufs=8, space="PSUM") as psum,
        ):
            # ---- input DMAs, in the order the PE needs them ----
            xt_tiles = []
            xoff = 0
            for j, csz in enumerate(csizes):
                xt = xpool.tile([P, KD * csz], bf, tag=f"xt_{j}")
                # 4 parallel sub-DMAs (k-pairs) so the load spreads queues
                for s in range(4):
                    nc.sync.dma_start(
                        xt[:, s * 2 * csz:(s + 1) * 2 * csz],
                        xt_d[:, xoff + s * 2 * csz:xoff + (s + 1) * 2 * csz],
                    )
                xt_tiles.append(xt)
                xoff += KD * csz
                if j == 0:
                    # w1 folded tiles: block b = q*2 + h covers f-quarter q,
                    # k-half h; free index = (k%4)*1024 + (f%8)*128 + col.
                    # 4 sub-DMAs per block (256KB each) spread the load
                    # across queues so the first f-quarter lands in ~6us.
                    w1_tiles = []
                    for b in range(8):
                        w = wpool.tile([P, F], bf, tag=f"w1_{b}")
                        for s in range(4):
                            nc.sync.dma_start(
                                w[:, s * D:(s + 1) * D],
                                w1t_d[b * P:(b + 1) * P, s * D:(s + 1) * D],
                            )
                        w1_tiles.append(w)
            w2_tiles = []
            for m in range(KF // 4):
                w = wpool.tile([P, 4 * D], bf, tag=f"w2_{m}")
                nc.sync.dma_start(w[:], w2t_d[m * P:(m + 1) * P, :])
                w2_tiles.append(w)

            def w1_slice(f, k):
                b = (f // 8) * 2 + k // 4
                o = (k % 4) * D + (f % 8) * P
                return w1_tiles[b][:, o:o + P]

            def w2_slice(kf, d):
                o = (kf % 4) * D + d * P
                return w2_tiles[kf // 4][:, o:o + P]

            def mm1_chunk(j, csz, tag):
                """matmul1 + gelu for one chunk -> 32 H^T tiles [128, csz]."""
                ht_tiles = [None] * KF
                for f in range(KF):
                    ps = psum.tile([P, NCHUNK], f32, tag="ps")
                    for k in range(KD):
                        nc.tensor.matmul(
                            ps[:, :csz],
                            w1_slice(f, k),
                            xt_tiles[j][:, k * csz:(k + 1) * csz],
                            start=(k == 0),
                            stop=(k == KD - 1),
                        )
                    ht = hpool.tile([P, csz], bf, tag=f"{tag}_{f}")
                    nc.scalar.activation(
                        ht[:], ps[:, :csz],
                        mybir.ActivationFunctionType.Gelu,
                    )
                    ht_tiles[f] = ht
                return ht_tiles

            def mm2_chunk(ht_tiles, csz, yoff):
                """matmul2 (Y^T) + copy + store for one chunk. Pairs of
                d-tiles share one yt tile -> one 2KB-packet store DMA."""
                yt = None
                for d in range(ND):
                    ps = psum.tile([P, NCHUNK], f32, tag="ps")
                    for kf in range(KF):
                        nc.tensor.matmul(
                            ps[:, :csz],
                            w2_slice(kf, d),
                            ht_tiles[kf][:],
                            start=(kf == 0),
                            stop=(kf == KF - 1),
                        )
                    if d % 2 == 0:
                        yt = ypool.tile([P, 2 * NCHUNK], bf, tag="y")
                    half = (d % 2) * csz
                    nc.vector.tensor_copy(
                        yt[:, half:half + csz], ps[:, :csz]
                    )
                    if d % 2 == 1:
                        o = yoff + (d - 1) * csz
                        nc.sync.dma_start(
                            y_d[:, o:o + 2 * csz], yt[:, :2 * csz]
                        )

            ycums = np.concatenate(
                [[0], np.cumsum([KD * c for c in csizes])[:-1]]
            ).astype(int)

            ht0 = mm1_chunk(0, csizes[0], "ht")
            mm2_chunk(ht0, csizes[0], ycums[0])
            if len(csizes) > 2:
                # remainder chunk's mm1 early: its gelu tail (ACT-bound)
                # hides under the following big chunk's matmul work
                hts = mm1_chunk(1, csizes[1], "hts")
                ht1 = mm1_chunk(2, csizes[2], "ht")
                mm2_chunk(hts, csizes[1], ycums[1])
                mm2_chunk(ht1, csizes[2], ycums[2])
                rest = range(3, len(csizes))
            else:
                rest = range(1, len(csizes))
            for j in rest:
                htj = mm1_chunk(j, csizes[j], "ht")
                mm2_chunk(htj, csizes[j], ycums[j])

    nc.finalize()
    _PROGRAM_CACHE[cap] = (nc, order)
    return _PROGRAM_CACHE[cap]


def _fold_xt(xt_full, chunks, order):
    """[D, cap] -> [128, 8*cap] per-chunk (k, c) layout, processing order."""
    starts = np.concatenate([[0], np.cumsum(chunks)[:-1]])
    blocks = []
    for i in order:
        off, csz = int(starts[i]), chunks[i]
        blk = xt_full[:, off:off + csz]                   # [1024, csz]
        blk = blk.reshape(KD, P, csz).transpose(1, 0, 2).reshape(P, KD * csz)
        blocks.append(blk)
    return np.ascontiguousarray(np.concatenate(blocks, axis=1))


def _fold_w1(w1t):
    """[D, F] -> [1024, 4096]; row block b = q*2 + h holds f-quarter q,
    k-half h, free index (k%4)*1024 + (f%8)*128 + col."""
    # w1t[k*128+p, f] -> out[(q*2+h)*128+p, kk*1024 + j*128 + c]
    a = w1t.reshape(2, 4, P, 4, 8, P)       # [h, kk, p, q, j, c]
    a = a.transpose(3, 0, 2, 1, 4, 5)       # [q, h, p, kk, j, c]
    return np.ascontiguousarray(a.reshape(D, F))


def _fold_w2(w2t):
    """[F, D] -> [1024, 4096]; row block m holds kf=4m..4m+3."""
    return np.ascontiguousarray(
        w2t.reshape(KF // 4, 4, P, D).transpose(0, 2, 1, 3).reshape(D, F)
    )


def _prepare(x, Wr, W1, W2):
    xf = np.ascontiguousarray(x.reshape(N, D), dtype=np.float32)
    idx, gates = _route(xf, Wr)

    tok_lists, gate_lists = [], []
    for e in range(E):
        hits = idx == e                        # [N, 2], at most one True/row
        toks = np.nonzero(hits.any(axis=1))[0]
        g = gates[hits]                        # row-major -> aligned with toks
        tok_lists.append(toks)
        gate_lists.append(g.astype(np.float32))

    counts = [len(t) for t in tok_lists]
    cap = max(counts)
    return tok_lists, gate_lists, counts, cap, xf


def kernel(x, Wr, W1, W2, _trace=False):
    x = np.asarray(x, dtype=np.float32)
    Wr = np.asarray(Wr, dtype=np.float32)
    W1 = np.asarray(W1, dtype=np.float32)
    W2 = np.asarray(W2, dtype=np.float32)

    tok_lists, gate_lists, counts, cap, xf = _prepare(x, Wr, W1, W2)
    nc, order = _build_program(cap)
    chunks = _chunks_of(cap)

    xb = xf.astype(BF16)
    W1b = W1.astype(BF16)
    W2b = W2.astype(BF16)

    in_maps = []
    for e in range(E):
        xe = np.zeros((cap, D), dtype=BF16)
        xe[:counts[e]] = xb[tok_lists[e]]
        in_maps.append({
            "xt": _fold_xt(xe.T, chunks, order),
            "w1t": _fold_w1(np.ascontiguousarray(W1b[e].T)),
            "w2t": _fold_w2(np.ascontiguousarray(W2b[e].T)),
        })

    res = run_bass_kernel_spmd(nc, in_maps, list(range(E)), trace=_trace)

    # unfold y: processed chunk j holds Y^T [D, csz] as [128, 8*csz]
    # (free index = d_tile*csz + c) at folded offset 8*cum_j; chunk j is
    # original chunk order[j] at token offset starts[order[j]].
    starts = np.concatenate([[0], np.cumsum(chunks)[:-1]]).astype(int)
    csizes = [chunks[i] for i in order]
    out = np.zeros((N, D), dtype=np.float32)
    for e in range(E):
        c = counts[e]
        yf = np.asarray(res.results[e]["y"])                  # [128, 8*cap]
        yt = np.empty((D, cap), dtype=yf.dtype)               # [D, cap]
        cum = 0
        for j, csz in enumerate(csizes):
            blk = yf[:, KD * cum:KD * (cum + csz)].reshape(P, ND, csz)
            o = starts[order[j]]
            yt[:, o:o + csz] = blk.transpose(1, 0, 2).reshape(D, csz)
            cum += csz
        y = yt[:, :c].astype(np.float32)
        out[tok_lists[e]] += y.T * gate_lists[e][:, None]

    out = out.reshape(B, T, D)
    if _trace:
        return out, res
    return out


# revision 26
# speedup vs baseline: 1.2219x; 1.0115x over previous
"""MoE layer (8 experts, top-2) on 8 Trainium2 NeuronCores.

Strategy (expert-parallel with host-side routing):
  - Router (0.01% of FLOPs) runs on host in float64: logits = x @ Wr.T,
    softmax, top-2, normalized gate weights.
  - Tokens are gathered per expert on host; core e runs expert e's FFN
    (gelu MLP 1024 -> 4096 -> 1024, bf16 in / fp32 accum / bf16 out) over
    the tokens routed to it, padded to the max expert load (the SPMD
    program is shared, so every core runs the max-count shape).
  - Host scatter-adds the gate-weighted expert outputs back into the full
    [B, T, D] fp32 output.

Device kernel per core (tokens processed in chunks of <=512):
  H^T tiles = gelu(W1T.T @ XT)   (matmul1: K=D, M=F-tile, N=token-chunk)
  Y^T tiles = W2T.T @ H^T        (matmul2: K=F, M=D-tile, N=token-chunk)
  Y is rounded to bf16 (matches the reference's bf16 expert output) and
  returned transposed [D, cap]; the host transposes back.

Performance notes:
  - DMA throughput is governed by the contiguous bytes per SBUF partition
    row (packet size): [128, 4096] bf16 tiles give 8KB packets (~300GB/s
    aggregate), [128, 512] only 1KB (~130GB/s). Inputs are re-laid-out on
    host so every load has >=2KB rows.
  - W1 is loaded as f-quarter x k-half tiles (folded on host to 8KB rows),
    so matmul1's first 8 f-tiles only need ~2.1MB before the PE can start
    (~6us) instead of waiting for all 8.4MB of W1.
  - The 47-token remainder chunk's matmul1 is emitted before the last big
    chunk so its ACT-bound gelu tail hides under matmul work.
  - matmul2 produces Y^T (d on partitions, tokens moving): the final
    partial token tile costs 47 columns instead of a 37%-filled PE pass.
"""

import numpy as np
import ml_dtypes

import concourse.mybir as mybir
import concourse.tile as tile
from concourse import bacc
from concourse.bass_utils import run_bass_kernel_spmd

BF16 = ml_dtypes.bfloat16

B, T, D, F, E = 2, 2048, 1024, 4096, 8
N = B * T
TOP_K = 2
P = 128
KD = D // P    # 8  k-tiles for matmul1
KF = F // P    # 32 k-tiles for matmul2
ND = D // P    # 8  output d-tiles for matmul2
NCHUNK = 512   # max token-chunk (moving free dim)


def _route(xf, Wr):
    """Top-2 routing in float64. Returns (idx [N,2], gates [N,2] fp32)."""
    logits = xf.astype(np.float64) @ Wr.T.astype(np.float64)
    logits -= logits.max(axis=-1, keepdims=True)
    p = np.exp(logits)
    p /= p.sum(axis=-1, keepdims=True)
    # top-2 (ties broken by lower index, same as jax.lax.top_k)
    order = np.argsort(-p, axis=-1, kind="stable")
    idx = order[:, :TOP_K]
    pw = np.take_along_axis(p, idx, axis=-1).astype(np.float32)
    gates = pw / pw.sum(axis=-1, keepdims=True)
    return idx, gates


def _chunks_of(cap):
    """Full NCHUNK chunks plus a remainder, e.g. 1071 -> [512, 512, 47]."""
    out = []
    c = cap
    while c > 0:
        s = min(NCHUNK, c)
        out.append(s)
        c -= s
    return out


_PROGRAM_CACHE = {}


def _build_program(cap):
    if cap in _PROGRAM_CACHE:
        return _PROGRAM_CACHE[cap]

    bf = mybir.dt.bfloat16
    f32 = mybir.dt.float32

    chunks = _chunks_of(cap)
    # processing (and xt layout) order: big chunk 0, remainder, rest
    order = [0] + list(range(len(chunks) - 1, 0, -1)) + []
    order = [0, len(chunks) - 1] + list(range(1, len(chunks) - 1))
    csizes = [chunks[i] for i in order]
    # token offset of each processed chunk within the gathered sequence
    starts = np.concatenate([[0], np.cumsum(chunks)[:-1]])
    offs = [int(starts[i]) for i in order]

    nc = bacc.Bacc()
    # xt: folded [128, 8*cap]; processed chunk j occupies columns
    # [8*sum(csizes[:j]), ...), free index inside a chunk = k*csz + c.
    xt_d = nc.declare_dram_parameter("xt", [P, KD * cap], bf, isOutput=False)
    w1t_d = nc.declare_dram_parameter("w1t", [2 * D, F // 2], bf,
                                      isOutput=False)
    # w2 folded [1024, 4096]; row block m holds kf=4m..4m+3, free = kk*D + d.
    w2t_d = nc.declare_dram_parameter("w2t", [D, F], bf, isOutput=False)
    # y folded like xt: chunk j at columns [8*xoff_j, ...), free index
    # inside a chunk = d_tile*csz + c (host unfolds).
    y_d = nc.declare_dram_parameter("y", [P, ND * cap], bf, isOutput=True)

    with tile.TileContext(nc) as tc:
        with (
            tc.tile_pool(name="wpool", bufs=1) as wpool,
            tc.tile_pool(name="xpool", bufs=1) as xpool,
            tc.tile_pool(name="hpool", bufs=1) as hpool,
            tc.tile_pool(name="ypool", bufs=4) as ypool,
            tc.tile_pool(name="psum", bufs=8, space="PSUM") as psum,
        ):
            # ---- input DMAs, in the order the PE needs them ----
            xt_tiles = []
            xoff = 0
            for j, csz in enumerate(csizes):
                xt = xpool.tile([P, KD * csz], bf, tag=f"xt_{j}")
                # 4 parallel sub-DMAs (k-pairs) so the load spreads queues
                for s in range(4):
                    nc.sync.dma_start(
                        xt[:, s * 2 * csz:(s + 1) * 2 * csz],
                        xt_d[:, xoff + s * 2 * csz:xoff + (s + 1) * 2 * csz],
                    )
                xt_tiles.append(xt)
                xoff += KD * csz
                if j == 0:
                    # w1 folded tiles: block b = o*2 + h covers f-eighth o
                    # (f-tiles 4o..4o+3), k-half h; free index =
                    # (k%4)*512 + (f%4)*128 + col. 2 sub-DMAs per block
                    # (256KB, 2KB rows): the first f-eighth (1MB, 4 DMAs
                    # on 4 queues) lets the PE start early.
                    w1_tiles = []
                    for b in range(16):
                        w = wpool.tile([P, F // 2], bf, tag=f"w1_{b}")
                        for s in range(2):
                            nc.sync.dma_start(
                                w[:, s * D:(s + 1) * D],
                                w1t_d[b * P:(b + 1) * P, s * D:(s + 1) * D],
                            )
                        w1_tiles.append(w)
            w2_tiles = []
            for m in range(KF // 4):
                w = wpool.tile([P, 4 * D], bf, tag=f"w2_{m}")
                nc.sync.dma_start(w[:], w2t_d[m * P:(m + 1) * P, :])
                w2_tiles.append(w)

            def w1_slice(f, k):
                b = (f // 4) * 2 + k // 4
                o = (k % 4) * (D // 2) + (f % 4) * P
                return w1_tiles[b][:, o:o + P]

            def w2_slice(kf, d):
                o = (kf % 4) * D + d * P
                return w2_tiles[kf // 4][:, o:o + P]

            def mm1_chunk(j, csz, tag):
                """matmul1 + gelu for one chunk -> 32 H^T tiles [128, csz]."""
                ht_tiles = [None] * KF
                for f in range(KF):
                    ps = psum.tile([P, NCHUNK], f32, tag="ps")
                    for k in range(KD):
                        nc.tensor.matmul(
                            ps[:, :csz],
                            w1_slice(f, k),
                            xt_tiles[j][:, k * csz:(k + 1) * csz],
                            start=(k == 0),
                            stop=(k == KD - 1),
                        )
                    ht = hpool.tile([P, csz], bf, tag=f"{tag}_{f}")
                    nc.scalar.activation(
                        ht[:], ps[:, :csz],
                        mybir.ActivationFunctionType.Gelu,
                    )
                    ht_tiles[f] = ht
                return ht_tiles

            def mm2_chunk(ht_tiles, csz, yoff):
                """matmul2 (Y^T) + copy + store for one chunk. Pairs of
                d-tiles share one yt tile -> one 2KB-packet store DMA."""
                yt = None
                for d in range(ND):
                    ps = psum.tile([P, NCHUNK], f32, tag="ps")
                    for kf in range(KF):
                        nc.tensor.matmul(
                            ps[:, :csz],
                            w2_slice(kf, d),
                            ht_tiles[kf][:],
                            start=(kf == 0),
                            stop=(kf == KF - 1),
                        )
                    if d % 2 == 0:
                        yt = ypool.tile([P, 2 * NCHUNK], bf, tag="y")
                    half = (d % 2) * csz
                    nc.vector.tensor_copy(
                        yt[:, half:half + csz], ps[:, :csz]
                    )
                    if d % 2 == 1:
                        o = yoff + (d - 1) * csz
                        nc.sync.dma_start(
                            y_d[:, o:o + 2 * csz], yt[:, :2 * csz]
                        )

            ycums = np.concatenate(
                [[0], np.cumsum([KD * c for c in csizes])[:-1]]
            ).astype(int)

            ht0 = mm1_chunk(0, csizes[0], "ht")
            mm2_chunk(ht0, csizes[0], ycums[0])
            if len(csizes) > 2:
                # remainder chunk's mm1 early: its gelu tail (ACT-bound)
                # hides under the following big chunk's matmul work
                hts = mm1_chunk(1, csizes[1], "hts")
                ht1 = mm1_chunk(2, csizes[2], "ht")
                # the remainder's mm2 goes LAST: a 12KB final store instead
                # of a 256KB one shortens the post-matmul tail
                mm2_chunk(ht1, csizes[2], ycums[2])
                mm2_chunk(hts, csizes[1], ycums[1])
                rest = range(3, len(csizes))
            else:
                rest = range(1, len(csizes))
            for j in rest:
                htj = mm1_chunk(j, csizes[j], "ht")
                mm2_chunk(htj, csizes[j], ycums[j])

    nc.finalize()
    _PROGRAM_CACHE[cap] = (nc, order)
    return _PROGRAM_CACHE[cap]


def _fold_xt(xt_full, chunks, order):
    """[D, cap] -> [128, 8*cap] per-chunk (k, c) layout, processing order."""
    starts = np.concatenate([[0], np.cumsum(chunks)[:-1]])
    blocks = []
    for i in order:
        off, csz = int(starts[i]), chunks[i]
        blk = xt_full[:, off:off + csz]                   # [1024, csz]
        blk = blk.reshape(KD, P, csz).transpose(1, 0, 2).reshape(P, KD * csz)
        blocks.append(blk)
    return np.ascontiguousarray(np.concatenate(blocks, axis=1))


def _fold_w1(w1t):
    """[D, F] -> [2048, 2048]; row block b = o*2 + h holds f-eighth o,
    k-half h, free index (k%4)*512 + (f%4)*128 + col."""
    # w1t[k*128+p, f] -> out[(o*2+h)*128+p, kk*512 + j*128 + c]
    a = w1t.reshape(2, 4, P, 8, 4, P)       # [h, kk, p, o, j, c]
    a = a.transpose(3, 0, 2, 1, 4, 5)       # [o, h, p, kk, j, c]
    return np.ascontiguousarray(a.reshape(2 * D, F // 2))


def _fold_w2(w2t):
    """[F, D] -> [1024, 4096]; row block m holds kf=4m..4m+3."""
    return np.ascontiguousarray(
        w2t.reshape(KF // 4, 4, P, D).transpose(0, 2, 1, 3).reshape(D, F)
    )


def _prepare(x, Wr, W1, W2):
    xf = np.ascontiguousarray(x.reshape(N, D), dtype=np.float32)
    idx, gates = _route(xf, Wr)

    tok_lists, gate_lists = [], []
    for e in range(E):
        hits = idx == e                        # [N, 2], at most one True/row
        toks = np.nonzero(hits.any(axis=1))[0]
        g = gates[hits]                        # row-major -> aligned with toks
        tok_lists.append(toks)
        gate_lists.append(g.astype(np.float32))

    counts = [len(t) for t in tok_lists]
    cap = max(counts)
    return tok_lists, gate_lists, counts, cap, xf


def kernel(x, Wr, W1, W2, _trace=False):
    x = np.asarray(x, dtype=np.float32)
    Wr = np.asarray(Wr, dtype=np.float32)
    W1 = np.asarray(W1, dtype=np.float32)
    W2 = np.asarray(W2, dtype=np.float32)

    tok_lists, gate_lists, counts, cap, xf = _prepare(x, Wr, W1, W2)
    nc, order = _build_program(cap)
    chunks = _chunks_of(cap)

    xb = xf.astype(BF16)
    W1b = W1.astype(BF16)
    W2b = W2.astype(BF16)

    in_maps = []
    for e in range(E):
        xe = np.zeros((cap, D), dtype=BF16)
        xe[:counts[e]] = xb[tok_lists[e]]
        in_maps.append({
            "xt": _fold_xt(xe.T, chunks, order),
            "w1t": _fold_w1(np.ascontiguousarray(W1b[e].T)),
            "w2t": _fold_w2(np.ascontiguousarray(W2b[e].T)),
        })

    res = run_bass_kernel_spmd(nc, in_maps, list(range(E)), trace=_trace)

    # unfold y: processed chunk j holds Y^T [D, csz] as [128, 8*csz]
    # (free index = d_tile*csz + c) at folded offset 8*cum_j; chunk j is
    # original chunk order[j] at token offset starts[order[j]].
    starts = np.concatenate([[0], np.cumsum(chunks)[:-1]]).astype(int)
    csizes = [chunks[i] for i in order]
    out = np.zeros((N, D), dtype=np.float32)
    for e in range(E):
        c = counts[e]
        yf = np.asarray(res.results[e]["y"])                  # [128, 8*cap]
        yt = np.empty((D, cap), dtype=yf.dtype)               # [D, cap]
        cum = 0
        for j, csz in enumerate(csizes):
            blk = yf[:, KD * cum:KD * (cum + csz)].reshape(P, ND, csz)
            o = starts[order[j]]
            yt[:, o:o + csz] = blk.transpose(1, 0, 2).reshape(D, csz)
            cum += csz
        y = yt[:, :c].astype(np.float32)
        out[tok_lists[e]] += y.T * gate_lists[e][:, None]

    out = out.reshape(B, T, D)
    if _trace:
        return out, res
    return out
